# revision 7
# baseline (speedup 1.0000x reference)
"""BatchHardTripletLoss on 8 Trainium2 NeuronCores.

Strategy (data parallel over rows; all reductions in squared-distance space;
sqrt is monotone so squared-space hardest-pos/neg selection is exact):

  Host: sort rows by label. Core c owns sorted rows [1024c, 1024c+1024).
  Columns (all 8192 candidates) are rotated per core so its own rows sit at
  fixed local columns [W/2, W/2+1024) -> every row-tile's same-class columns
  fall in a fixed local window => one SPMD program for all 8 cores.

  Two device pipelines per core, split by column region:
   1) Row path (local cols [0,1536) u [3584,8192), includes the class band):
      TensorE assembles psum[i,j] = sq_j - 2 x_i.x_j + BAND*same(i,j) with
      three matmuls per chunk: f32r feats matmul, a K=1 rank-1 matmul
      broadcasting sq_j from a [1,8192] row, and (chunk 0 only) fp8e5
      class-indicator matmuls adding BAND=2^15 to same-class pairs.
      VectorE then does ONE min-reduce per 1536 chunk (hardest-neg) and one
      max-reduce over the band window (hardest-pos; host subtracts BAND).
   2) Transposed path (local cols [1536,3584), guaranteed band-free):
      TensorE: psum[j,i] = x_j.x_i for 16 j-tiles x all 1024 own rows;
      ScalarE: tbuf = 2*psum - sq_j (per-partition bias, Identity act);
      GpSimd:  partition_all_reduce(max) over the 128 j's -> per-jt row
      maxima, shipped to host which negates (min = -max(-t)) and combines.

  Host epilogue: + sq_i, clamp, sqrt (eps rule), validity from label counts
  (self-inclusion in hardest-pos is harmless: singleton classes are invalid
  by count), margin + masked mean in fp32.
"""

import numpy as np

N = 8192
D = 128
MARGIN = 0.3
NCORES = 8
ROWS_PER_CORE = N // NCORES          # 1024
RT_PER_CORE = ROWS_PER_CORE // 128   # 8 row-tiles
RW = 1536                            # row-path psum chunk width (3 banks)
TR0 = 1536                           # transposed region start (local cols)
TRN = 16                             # transposed j-tiles (128 each)
ROW_GROUPS = [(0, 1536), (3584, 5120), (5120, 6656), (6656, 8192)]
MMN = 512
BAND = 32768.0                       # fp8e5-exact mask magnitude (2^15)
NG_BY_W = {192: 2, 256: 2, 384: 3, 512: 4}

_PROGRAM_CACHE = {}


def _build_program(W):
    import concourse.mybir as mybir
    import concourse.bass_isa as bass_isa
    from concourse import bacc
    from concourse.tile import TileContext

    F32 = mybir.dt.float32
    F32R = mybir.dt.float32r
    FP8 = mybir.dt.float8e5
    NG = NG_BY_W[W]

    nc = bacc.Bacc("TRN2", target_bir_lowering=False, debug=False,
                   num_devices=NCORES)

    featsT_d = nc.dram_tensor("featsT", [D, N], F32R, kind="ExternalInput")
    rows2_d = nc.dram_tensor("rows2", [D, ROWS_PER_CORE], F32R,
                             kind="ExternalInput")
    one1_d = nc.dram_tensor("one1", [1, 128], F32R, kind="ExternalInput")
    sqrow_d = nc.dram_tensor("sqrow", [1, N], F32R, kind="ExternalInput")
    negsq_d = nc.dram_tensor("negsq", [D, TRN], F32, kind="ExternalInput")
    bstat_d = nc.dram_tensor("bstat", [D, RT_PER_CORE * NG * 128], FP8,
                             kind="ExternalInput")
    bmov_d = nc.dram_tensor("bmov", [D, RT_PER_CORE * NG * W], FP8,
                            kind="ExternalInput")
    neg_out_d = nc.dram_tensor("neg_out", [D, RT_PER_CORE], F32,
                               kind="ExternalOutput")
    pos_out_d = nc.dram_tensor("pos_out", [D, RT_PER_CORE], F32,
                               kind="ExternalOutput")
    gneg_out_d = nc.dram_tensor("gneg_out", [TRN // 4, 4 * ROWS_PER_CORE], F32,
                                kind="ExternalOutput")

    with TileContext(nc) as tc:
        with (
            tc.tile_pool(name="big", bufs=1) as big,
            tc.tile_pool(name="vps", bufs=2, space="PSUM") as vps_pool,
            tc.tile_pool(name="tps", bufs=2, space="PSUM") as tps_pool,
            tc.tile_pool(name="tb", bufs=3) as tb_pool,
            tc.tile_pool(name="scr", bufs=2) as scr_pool,
            tc.tile_pool(name="small", bufs=1) as small,
        ):
            featsT = big.tile([D, N], F32R, tag="featsT")
            rows2 = big.tile([D, ROWS_PER_CORE], F32R, tag="rows2")
            one1 = small.tile([1, 128], F32R, tag="one1")
            sqrow = small.tile([1, N], F32R, tag="sqrow")
            negsq = small.tile([D, TRN], F32, tag="negsq")
            bstat = big.tile([D, RT_PER_CORE * NG * 128], FP8, tag="bstat")
            bmov = big.tile([D, RT_PER_CORE * NG * W], FP8, tag="bmov")
            neg_sb = small.tile([D, RT_PER_CORE], F32, tag="neg_sb")
            pos_sb = small.tile([D, RT_PER_CORE], F32, tag="pos_sb")

            # critical-path first; spread issue across the 3 DMA-capable
            # queues (sync / scalar / gpsimd sequencers feed the hw queues)
            nc.sync.dma_start(rows2[:, :], rows2_d[:, :])
            nc.scalar.dma_start(one1[:, :], one1_d[:, :])
            nc.scalar.dma_start(sqrow[:, :], sqrow_d[:, :])
            nc.gpsimd.dma_start(featsT[:, 0:1024], featsT_d[:, 0:1024])
            nc.sync.dma_start(featsT[:, 1024:2048], featsT_d[:, 1024:2048])
            nc.scalar.dma_start(negsq[:, :], negsq_d[:, :])
            nc.gpsimd.dma_start(bstat[:, :], bstat_d[:, :])
            nc.sync.dma_start(bmov[:, :], bmov_d[:, :])
            for ch in range(2, 8):
                q = (nc.sync, nc.scalar, nc.gpsimd)[ch % 3]
                q.dma_start(featsT[:, ch * 1024:(ch + 1) * 1024],
                            featsT_d[:, ch * 1024:(ch + 1) * 1024])

            def emit_transposed(q):
                # 4 j-tiles -> one concatenated tbuf -> one partition_all_reduce
                tbuf = tb_pool.tile([D, 4 * ROWS_PER_CORE], F32, tag="tbuf",
                                    name=f"tbuf{q}")
                for u in range(4):
                    jt = 4 * q + u
                    lhsT = featsT[:, TR0 + 128 * jt:TR0 + 128 * (jt + 1)]
                    for h in range(2):
                        ps_t = tps_pool.tile([D, MMN], F32, tag="ps_t",
                                             name=f"ps_t{jt}_{h}")
                        nc.tensor.matmul(
                            ps_t[:, :], lhsT,
                            featsT[:, W // 2 + h * MMN:W // 2 + (h + 1) * MMN],
                            start=True, stop=True)
                        nc.scalar.activation(
                            tbuf[:, u * ROWS_PER_CORE + h * MMN:
                                 u * ROWS_PER_CORE + (h + 1) * MMN],
                            ps_t[:, :],
                            mybir.ActivationFunctionType.Identity,
                            bias=negsq[:, jt:jt + 1], scale=2.0)
                gout = tb_pool.tile([D, 4 * ROWS_PER_CORE], F32, tag="gout",
                                    name=f"gout{q}")
                nc.gpsimd.partition_all_reduce(
                    gout[:, :], tbuf[:, :], 128, bass_isa.ReduceOp.max)
                nc.sync.dma_start(gneg_out_d[q:q + 1, :], gout[0:1, :])

            for lt in range(RT_PER_CORE):
                if lt % 2 == 0 and lt < 6:
                    emit_transposed(lt // 2)
                elif lt == 5:
                    emit_transposed(3)
                lhsT = rows2[:, 128 * lt:128 * (lt + 1)]
                partials = scr_pool.tile([D, 4], F32, tag="partials",
                                         name=f"partials{lt}")
                w0 = 128 * lt + 64
                for g, (c0g, c1g) in enumerate(ROW_GROUPS):
                    ps = vps_pool.tile([D, RW], F32, tag="ps",
                                       name=f"ps{lt}_{g}")
                    for k in range(RW // MMN):
                        nc.tensor.matmul(
                            ps[:, k * MMN:(k + 1) * MMN], lhsT,
                            featsT[:, c0g + k * MMN:c0g + (k + 1) * MMN],
                            start=True, stop=False)
                        nc.tensor.matmul(
                            ps[:, k * MMN:(k + 1) * MMN], one1[:, :],
                            sqrow[:, c0g + k * MMN:c0g + (k + 1) * MMN],
                            start=False, stop=True)
                    if g == 0:
                        # band pieces split at psum bank boundaries
                        pieces = []
                        for k in range(3):
                            a = max(w0, MMN * k)
                            b = min(w0 + W, MMN * (k + 1))
                            if a < b:
                                pieces.append((a, b))
                        for bg in range(NG):
                            soff = (lt * NG + bg) * 128
                            moff = (lt * NG + bg) * W
                            for (a, b) in pieces:
                                nc.tensor.matmul(
                                    ps[:, a:b],
                                    bstat[:, soff:soff + 128],
                                    bmov[:, moff + a - w0:moff + b - w0],
                                    start=False, stop=True,
                                    skip_group_check=True)
                        nc.vector.tensor_reduce(
                            pos_sb[:, lt:lt + 1], ps[:, w0:w0 + W],
                            axis=mybir.AxisListType.X, op=mybir.AluOpType.max)
                    nc.vector.tensor_reduce(
                        partials[:, g:g + 1], ps[:, :],
                        axis=mybir.AxisListType.X, op=mybir.AluOpType.min)
                nc.vector.tensor_reduce(
                    neg_sb[:, lt:lt + 1], partials[:, 0:4],
                    axis=mybir.AxisListType.X, op=mybir.AluOpType.min)

            nc.sync.dma_start(neg_out_d[:, :], neg_sb[:, :])
            nc.sync.dma_start(pos_out_d[:, :], pos_sb[:, :])

    nc.compile()
    return nc


def kernel(feats, labels):
    import ml_dtypes
    from concourse.bass_utils import run_bass_kernel_spmd

    feats = np.asarray(feats, dtype=np.float32)
    labels_np = np.asarray(labels).astype(np.int64)

    order = np.argsort(labels_np, kind="stable")
    feats_s = feats[order]
    labels_s = labels_np[order]

    counts = np.bincount(labels_s, minlength=max(int(labels_s.max()) + 1, 1))
    mc = int(counts.max())
    if mc <= 33:
        W = 192
    elif mc <= 65:
        W = 256
    elif mc <= 129:
        W = 384
    elif mc <= 193:
        W = 512
    else:
        raise ValueError(f"class of size {mc} exceeds supported band window")
    NG = NG_BY_W[W]

    if W not in _PROGRAM_CACHE:
        _PROGRAM_CACHE[W] = _build_program(W)
    nc = _PROGRAM_CACHE[W]

    sq = np.einsum("nd,nd->n", feats_s.astype(np.float64),
                   feats_s.astype(np.float64)).astype(np.float32)
    one1_np = np.ones((1, 128), dtype=np.float32)

    in_maps = []
    for c in range(NCORES):
        rot = (ROWS_PER_CORE * c - W // 2) % N
        loc = (rot + np.arange(N)) % N          # local col -> global sorted row
        featsT_c = np.ascontiguousarray(feats_s[loc].T)
        rows2_c = np.ascontiguousarray(
            (-2.0 * feats_s[ROWS_PER_CORE * c:ROWS_PER_CORE * (c + 1)]).T)
        sq_loc = sq[loc]
        sqrow_c = np.ascontiguousarray(sq_loc[None, :])
        negsq_c = np.ascontiguousarray(
            -sq_loc[TR0:TR0 + TRN * 128].reshape(TRN, 128).T)
        bstat_c = np.zeros((D, RT_PER_CORE * NG * 128), dtype=np.float32)
        bmov_c = np.zeros((D, RT_PER_CORE * NG * W), dtype=np.float32)
        for lt in range(RT_PER_CORE):
            rows_lab = labels_s[ROWS_PER_CORE * c + 128 * lt:
                                ROWS_PER_CORE * c + 128 * (lt + 1)]
            w0 = 128 * lt + 64
            win_lab = labels_s[loc[w0:w0 + W]]
            # rank classes by first appearance in the (sorted) window
            uniq, first = np.unique(win_lab, return_index=True)
            rank_of = {int(cls): r for r, cls in
                       enumerate(uniq[np.argsort(first)])}
            row_rank = np.array([rank_of[int(l)] for l in rows_lab])
            col_rank = np.array([rank_of[int(l)] for l in win_lab])
            for bg in range(NG):
                soff = (lt * NG + bg) * 128
                moff = (lt * NG + bg) * W
                rsel = (row_rank >= 128 * bg) & (row_rank < 128 * (bg + 1))
                csel = (col_rank >= 128 * bg) & (col_rank < 128 * (bg + 1))
                bstat_c[row_rank[rsel] - 128 * bg,
                        soff + np.arange(128)[rsel]] = BAND
                bmov_c[col_rank[csel] - 128 * bg,
                       moff + np.arange(W)[csel]] = 1.0
        in_maps.append({
            "featsT": featsT_c,
            "rows2": rows2_c,
            "one1": one1_np,
            "sqrow": sqrow_c,
            "negsq": negsq_c,
            "bstat": bstat_c.astype(ml_dtypes.float8_e5m2),
            "bmov": bmov_c.astype(ml_dtypes.float8_e5m2),
        })

    res = run_bass_kernel_spmd(nc, in_maps, core_ids=list(range(NCORES)))

    neg_raw = np.empty(N, dtype=np.float32)
    pos_raw = np.empty(N, dtype=np.float32)
    for c in range(NCORES):
        base = ROWS_PER_CORE * c
        nr = res.results[c]["neg_out"].T.reshape(ROWS_PER_CORE)
        tr = -res.results[c]["gneg_out"].reshape(TRN, ROWS_PER_CORE).max(axis=0)
        neg_raw[base:base + ROWS_PER_CORE] = np.minimum(nr, tr)
        pos_raw[base:base + ROWS_PER_CORE] = \
            res.results[c]["pos_out"].T.reshape(ROWS_PER_CORE) - \
            np.float32(BAND)

    hn_sq = np.maximum(neg_raw + sq, 0.0).astype(np.float32)
    hp_sq = np.maximum(pos_raw + sq, 0.0).astype(np.float32)
    eps = np.float32(1e-12)
    hn = np.where(hn_sq > eps, np.sqrt(hn_sq), np.float32(0.0))
    hp = np.where(hp_sq > eps, np.sqrt(hp_sq), np.float32(0.0))

    cnt_per_row = counts[labels_s]
    valid = (cnt_per_row >= 2) & (cnt_per_row < N)
    diff = np.where(valid, hp - hn, np.float32(0.0))
    per_row = np.maximum(diff + np.float32(MARGIN), np.float32(0.0))
    per_row = np.where(valid, per_row, np.float32(0.0)).astype(np.float32)
    cnt = np.float32(valid.sum())
    if cnt > 0:
        loss = np.float32(per_row.sum(dtype=np.float32) / max(cnt, np.float32(1.0)))
    else:
        loss = np.float32(0.0)
    return np.float32(loss)


# revision 16
# speedup vs baseline: 1.8968x; 1.8968x over previous
"""BatchHardTripletLoss on 8 Trainium2 NeuronCores.

Strategy (data parallel over rows; all reductions in squared-distance space;
sqrt is monotone so squared-space hardest-pos/neg selection is exact):

  Host: sort rows by label. Core c owns sorted rows [1024c, 1024c+1024).
  Columns (all 8192 candidates) are rotated per core so its own rows sit at
  fixed local columns [W/2, W/2+1024) -> every row-tile's same-class columns
  fall in a fixed local window => one SPMD program for all 8 cores.

  Two device pipelines per core, split by column region:
   1) Row path (local cols [0,1536) u [3584,8192), includes the class band):
      TensorE assembles psum[i,j] = sq_j - 2 x_i.x_j + BAND*same(i,j) with
      three matmuls per chunk: f32r feats matmul, a K=1 rank-1 matmul
      broadcasting sq_j from a [1,8192] row, and (chunk 0 only) fp8e5
      class-indicator matmuls adding BAND=2^15 to same-class pairs.
      VectorE then does ONE min-reduce per 1536 chunk (hardest-neg) and one
      max-reduce over the band window (hardest-pos; host subtracts BAND).
   2) Transposed path (local cols [1536,3584), guaranteed band-free):
      TensorE: psum[j,i] = x_j.x_i for 16 j-tiles x all 1024 own rows;
      ScalarE: tbuf = 2*psum - sq_j (per-partition bias, Identity act);
      GpSimd:  partition_all_reduce(max) over the 128 j's -> per-jt row
      maxima, shipped to host which negates (min = -max(-t)) and combines.

  Host epilogue: + sq_i, clamp, sqrt (eps rule), validity from label counts
  (self-inclusion in hardest-pos is harmless: singleton classes are invalid
  by count), margin + masked mean in fp32.
"""

import numpy as np

N = 8192
D = 128
MARGIN = 0.3
NCORES = 8
ROWS_PER_CORE = N // NCORES          # 1024
RT_PER_CORE = ROWS_PER_CORE // 128   # 8 row-tiles
RW = 1536                            # row-path psum chunk width (3 banks)
TR0 = 1536                           # transposed region start (local cols)
TRN = 16                             # transposed j-tiles (128 each)
ROW_GROUPS = [(0, 1536), (3584, 5120), (5120, 6656), (6656, 8192)]
MMN = 512
BAND = 32768.0                       # fp8e5-exact mask magnitude (2^15)
NG_BY_W = {192: 2, 256: 2, 384: 3, 512: 4}

_PROGRAM_CACHE = {}


def _build_program(W):
    import concourse.mybir as mybir
    import concourse.bass_isa as bass_isa
    from concourse import bacc
    from concourse.tile import TileContext

    F32 = mybir.dt.float32
    F32R = mybir.dt.float32r
    FP8 = mybir.dt.float8e5
    NG = NG_BY_W[W]

    nc = bacc.Bacc("TRN2", target_bir_lowering=False, debug=False,
                   num_devices=NCORES)

    F16 = mybir.dt.float16
    featsT_d = nc.dram_tensor("featsT", [D, N], F32R, kind="ExternalInput")
    rows2_d = nc.dram_tensor("rows2", [D, ROWS_PER_CORE], F32R,
                             kind="ExternalInput")
    ones16_d = nc.dram_tensor("ones16", [D, 128], F16, kind="ExternalInput")
    sqb16_d = nc.dram_tensor("sqb16", [D, 4 * RW], F16, kind="ExternalInput")
    negsq_d = nc.dram_tensor("negsq", [D, TRN], F32, kind="ExternalInput")
    bstat_d = nc.dram_tensor("bstat", [D, RT_PER_CORE * NG * 128], FP8,
                             kind="ExternalInput")
    bmov_d = nc.dram_tensor("bmov", [D, RT_PER_CORE * NG * W], FP8,
                            kind="ExternalInput")
    neg_out_d = nc.dram_tensor("neg_out", [D, RT_PER_CORE], F32,
                               kind="ExternalOutput")
    pos_out_d = nc.dram_tensor("pos_out", [D, RT_PER_CORE], F32,
                               kind="ExternalOutput")
    gneg_out_d = nc.dram_tensor("gneg_out", [TRN // 4, 4 * ROWS_PER_CORE], F32,
                                kind="ExternalOutput")

    with TileContext(nc) as tc:
        with (
            tc.tile_pool(name="big", bufs=1) as big,
            tc.tile_pool(name="vps", bufs=2, space="PSUM") as vps_pool,
            tc.tile_pool(name="tps", bufs=2, space="PSUM") as tps_pool,
            tc.tile_pool(name="tb", bufs=3) as tb_pool,
            tc.tile_pool(name="scr", bufs=2) as scr_pool,
            tc.tile_pool(name="small", bufs=1) as small,
        ):
            featsT = big.tile([D, N], F32R, tag="featsT")
            rows2 = big.tile([D, ROWS_PER_CORE], F32R, tag="rows2")
            ones16 = small.tile([D, 128], F16, tag="ones16")
            sqb16 = big.tile([D, 4 * RW], F16, tag="sqb16")
            negsq = small.tile([D, TRN], F32, tag="negsq")
            bstat = big.tile([D, RT_PER_CORE * NG * 128], FP8, tag="bstat")
            bmov = big.tile([D, RT_PER_CORE * NG * W], FP8, tag="bmov")
            neg_sb = small.tile([D, RT_PER_CORE], F32, tag="neg_sb")
            pos_sb = small.tile([D, RT_PER_CORE], F32, tag="pos_sb")

            # critical-path first; spread issue across the 3 DMA-capable
            # queues (sync / scalar / gpsimd sequencers feed the hw queues)
            nc.sync.dma_start(rows2[:, :], rows2_d[:, :])
            nc.scalar.dma_start(ones16[:, :], ones16_d[:, :])
            nc.scalar.dma_start(sqb16[:, 0:2 * RW], sqb16_d[:, 0:2 * RW])
            nc.gpsimd.dma_start(featsT[:, 0:1024], featsT_d[:, 0:1024])
            nc.sync.dma_start(featsT[:, 1024:2048], featsT_d[:, 1024:2048])
            nc.scalar.dma_start(negsq[:, :], negsq_d[:, :])
            nc.gpsimd.dma_start(bstat[:, :], bstat_d[:, :])
            nc.sync.dma_start(bmov[:, :], bmov_d[:, :])
            nc.gpsimd.dma_start(sqb16[:, 2 * RW:4 * RW],
                                sqb16_d[:, 2 * RW:4 * RW])
            for ch in range(2, 8):
                q = (nc.sync, nc.scalar, nc.gpsimd)[ch % 3]
                q.dma_start(featsT[:, ch * 1024:(ch + 1) * 1024],
                            featsT_d[:, ch * 1024:(ch + 1) * 1024])

            def emit_transposed(q):
                # 4 j-tiles -> one concatenated tbuf -> one partition_all_reduce
                tbuf = tb_pool.tile([D, 4 * ROWS_PER_CORE], F32, tag="tbuf",
                                    name=f"tbuf{q}")
                for u in range(4):
                    jt = 4 * q + u
                    lhsT = featsT[:, TR0 + 128 * jt:TR0 + 128 * (jt + 1)]
                    for h in range(2):
                        ps_t = tps_pool.tile([D, MMN], F32, tag="ps_t",
                                             name=f"ps_t{jt}_{h}")
                        nc.tensor.matmul(
                            ps_t[:, :], lhsT,
                            featsT[:, W // 2 + h * MMN:W // 2 + (h + 1) * MMN],
                            start=True, stop=True)
                        nc.scalar.activation(
                            tbuf[:, u * ROWS_PER_CORE + h * MMN:
                                 u * ROWS_PER_CORE + (h + 1) * MMN],
                            ps_t[:, :],
                            mybir.ActivationFunctionType.Identity,
                            bias=negsq[:, jt:jt + 1], scale=2.0)
                gout = tb_pool.tile([D, 4 * ROWS_PER_CORE], F32, tag="gout",
                                    name=f"gout{q}")
                nc.gpsimd.partition_all_reduce(
                    gout[:, :], tbuf[:, :], 128, bass_isa.ReduceOp.max)
                nc.sync.dma_start(gneg_out_d[q:q + 1, :], gout[0:1, :])

            partials = small.tile([D, RT_PER_CORE, 4], F32, tag="partials")

            for lt in range(RT_PER_CORE):
                if lt % 2 == 0 and lt < 6:
                    emit_transposed(lt // 2)
                elif lt == 5:
                    emit_transposed(3)
                lhsT = rows2[:, 128 * lt:128 * (lt + 1)]
                w0 = 128 * lt + 64
                # process chunks in pairs; within a pair run all rows2
                # matmuls back-to-back, then all sq matmuls (fewer PE
                # stationary switches)
                for half in range(2):
                    gs = (2 * half, 2 * half + 1)
                    pss = {}
                    for g in gs:
                        c0g = ROW_GROUPS[g][0]
                        ps = vps_pool.tile([D, RW], F32, tag="ps",
                                           name=f"ps{lt}_{g}")
                        pss[g] = ps
                        for k in range(RW // MMN):
                            nc.tensor.matmul(
                                ps[:, k * MMN:(k + 1) * MMN], lhsT,
                                featsT[:, c0g + k * MMN:c0g + (k + 1) * MMN],
                                start=True, stop=False)
                    for g in gs:
                        c0g = ROW_GROUPS[g][0]
                        ps = pss[g]
                        for k in range(RW // MMN):
                            nc.tensor.matmul(
                                ps[:, k * MMN:(k + 1) * MMN], ones16[:, :],
                                sqb16[:, g * RW + k * MMN:
                                      g * RW + (k + 1) * MMN],
                                start=False, stop=True)
                    if half == 0:
                        ps = pss[0]
                        # band pieces split at psum bank boundaries
                        pieces = []
                        for k in range(3):
                            a = max(w0, MMN * k)
                            b = min(w0 + W, MMN * (k + 1))
                            if a < b:
                                pieces.append((a, b))
                        for bg in range(NG):
                            soff = (lt * NG + bg) * 128
                            moff = (lt * NG + bg) * W
                            for (a, b) in pieces:
                                nc.tensor.matmul(
                                    ps[:, a:b],
                                    bstat[:, soff:soff + 128],
                                    bmov[:, moff + a - w0:moff + b - w0],
                                    start=False, stop=True,
                                    skip_group_check=True)
                        nc.vector.tensor_reduce(
                            pos_sb[:, lt:lt + 1], ps[:, w0:w0 + W],
                            axis=mybir.AxisListType.X, op=mybir.AluOpType.max)
                    for g in gs:
                        nc.vector.tensor_reduce(
                            partials[:, lt, g:g + 1],
                            pss[g][:, :],
                            axis=mybir.AxisListType.X, op=mybir.AluOpType.min)

            # one batched [128, 8, 4] -> [128, 8] min over all row-tiles
            nc.vector.tensor_reduce(
                neg_sb[:, :], partials[:, :, :],
                axis=mybir.AxisListType.X, op=mybir.AluOpType.min)

            nc.sync.dma_start(neg_out_d[:, :], neg_sb[:, :])
            nc.sync.dma_start(pos_out_d[:, :], pos_sb[:, :])

    nc.compile()
    return nc


def kernel(feats, labels):
    import ml_dtypes
    from concourse.bass_utils import run_bass_kernel_spmd

    feats = np.asarray(feats, dtype=np.float32)
    labels_np = np.asarray(labels).astype(np.int64)

    order = np.argsort(labels_np, kind="stable")
    feats_s = feats[order]
    labels_s = labels_np[order]

    counts = np.bincount(labels_s, minlength=max(int(labels_s.max()) + 1, 1))
    mc = int(counts.max())
    if mc <= 33:
        W = 192
    elif mc <= 65:
        W = 256
    elif mc <= 129:
        W = 384
    elif mc <= 193:
        W = 512
    else:
        raise ValueError(f"class of size {mc} exceeds supported band window")
    NG = NG_BY_W[W]

    if W not in _PROGRAM_CACHE:
        _PROGRAM_CACHE[W] = _build_program(W)
    nc = _PROGRAM_CACHE[W]

    sq = np.einsum("nd,nd->n", feats_s.astype(np.float64),
                   feats_s.astype(np.float64)).astype(np.float32)
    ones16_np = np.ones((D, 128), dtype=np.float16)
    rp_cols = np.concatenate([np.arange(a, b) for a, b in ROW_GROUPS])

    in_maps = []
    for c in range(NCORES):
        rot = (ROWS_PER_CORE * c - W // 2) % N
        loc = (rot + np.arange(N)) % N          # local col -> global sorted row
        featsT_c = np.ascontiguousarray(feats_s[loc].T)
        rows2_c = np.ascontiguousarray(
            (-2.0 * feats_s[ROWS_PER_CORE * c:ROWS_PER_CORE * (c + 1)]).T)
        sq_loc = sq[loc]
        sqb16_c = np.ascontiguousarray(np.broadcast_to(
            (sq_loc[rp_cols] / 128.0)[None, :], (D, 4 * RW))
        ).astype(np.float16)
        negsq_c = np.ascontiguousarray(
            -sq_loc[TR0:TR0 + TRN * 128].reshape(TRN, 128).T)
        bstat_c = np.zeros((D, RT_PER_CORE * NG * 128), dtype=np.float32)
        bmov_c = np.zeros((D, RT_PER_CORE * NG * W), dtype=np.float32)
        for lt in range(RT_PER_CORE):
            rows_lab = labels_s[ROWS_PER_CORE * c + 128 * lt:
                                ROWS_PER_CORE * c + 128 * (lt + 1)]
            w0 = 128 * lt + 64
            win_lab = labels_s[loc[w0:w0 + W]]
            # rank classes by first appearance in the (sorted) window
            uniq, first = np.unique(win_lab, return_index=True)
            rank_of = {int(cls): r for r, cls in
                       enumerate(uniq[np.argsort(first)])}
            row_rank = np.array([rank_of[int(l)] for l in rows_lab])
            col_rank = np.array([rank_of[int(l)] for l in win_lab])
            for bg in range(NG):
                soff = (lt * NG + bg) * 128
                moff = (lt * NG + bg) * W
                rsel = (row_rank >= 128 * bg) & (row_rank < 128 * (bg + 1))
                csel = (col_rank >= 128 * bg) & (col_rank < 128 * (bg + 1))
                bstat_c[row_rank[rsel] - 128 * bg,
                        soff + np.arange(128)[rsel]] = BAND
                bmov_c[col_rank[csel] - 128 * bg,
                       moff + np.arange(W)[csel]] = 1.0
        in_maps.append({
            "featsT": featsT_c,
            "rows2": rows2_c,
            "ones16": ones16_np,
            "sqb16": sqb16_c,
            "negsq": negsq_c,
            "bstat": bstat_c.astype(ml_dtypes.float8_e5m2),
            "bmov": bmov_c.astype(ml_dtypes.float8_e5m2),
        })

    res = run_bass_kernel_spmd(nc, in_maps, core_ids=list(range(NCORES)))

    neg_raw = np.empty(N, dtype=np.float32)
    pos_raw = np.empty(N, dtype=np.float32)
    for c in range(NCORES):
        base = ROWS_PER_CORE * c
        nr = res.results[c]["neg_out"].T.reshape(ROWS_PER_CORE)
        tr = -res.results[c]["gneg_out"].reshape(TRN, ROWS_PER_CORE).max(axis=0)
        neg_raw[base:base + ROWS_PER_CORE] = np.minimum(nr, tr)
        pos_raw[base:base + ROWS_PER_CORE] = \
            res.results[c]["pos_out"].T.reshape(ROWS_PER_CORE) - \
            np.float32(BAND)

    hn_sq = np.maximum(neg_raw + sq, 0.0).astype(np.float32)
    hp_sq = np.maximum(pos_raw + sq, 0.0).astype(np.float32)
    eps = np.float32(1e-12)
    hn = np.where(hn_sq > eps, np.sqrt(hn_sq), np.float32(0.0))
    hp = np.where(hp_sq > eps, np.sqrt(hp_sq), np.float32(0.0))

    cnt_per_row = counts[labels_s]
    valid = (cnt_per_row >= 2) & (cnt_per_row < N)
    diff = np.where(valid, hp - hn, np.float32(0.0))
    per_row = np.maximum(diff + np.float32(MARGIN), np.float32(0.0))
    per_row = np.where(valid, per_row, np.float32(0.0)).astype(np.float32)
    cnt = np.float32(valid.sum())
    if cnt > 0:
        loss = np.float32(per_row.sum(dtype=np.float32) / max(cnt, np.float32(1.0)))
    else:
        loss = np.float32(0.0)
    return np.float32(loss)
